# revision 9
# baseline (speedup 1.0000x reference)
"""Masked causal attention (B=2, T=2048, C=1024, N=16 heads, D=64) on 8 TRN2 cores.

Sharding: core c handles batch c//4 and head quad c%4 (heads 4g..4g+3, a
contiguous 256-channel block).  Per core: Q/K/V projections for its 256
columns over its batch's 2048 rows, causal-masked softmax attention for its
4 heads, and the partial output projection (Wo rows for its channels).
The host sums the 4 fp16 partials per batch and adds bo.

This quarters both big IO tensors vs head-only sharding (src per core is
one batch; the out partial is [2048, 1024] fp16), which matters because the
per-call cost of the SPMD executable scales with output-buffer bytes on
top of the device exec time.

Per-core dataflow (fp16 matmuls for proj/scores/o-proj; exp + V in bf16;
contraction always on partitions):
  srct [4, 128, 8, 512] fp16 chunks -> Q^T, K^T [128, 2, T] fp16 via
  lhsT=W-slices; V^T chunks are PE-transposed into v [s, head-major 65-col
  blocks] bf16 (64 v-cols + a ones column per head).
  scores^T [s, t] = (K^T 64-row slice).T @ Q^T chunk (contraction d=64);
  fully-masked column ranges of diagonal blocks are not computed; the
  128-wide straddling sub-block gets an additive -3e4 mask.
  attn_out^T + softmax denominator from one matmul per s-tile:
  lhsT = [v_head | ones] (65 cols) -> psum rows 0..63 = unnorm out^T,
  row 64 = sum of exp.  Normalization: reciprocal + gpsimd
  partition_broadcast of the denominator row, DVE multiply.
  o-proj: psum[t, c] = aoT.T @ Wo_slice (256-channel partial), staged to a
  [128, 1024] fp16 tile, one 256 KB DMA per 128 rows.
"""

import sys

sys.path.insert(0, "/opt/trn_rl_repo")

import numpy as np

B, T, C = 2, 2048, 1024
NHEADS = 16
D = 64
P = 128            # partitions
KC = C // P        # 8 contraction tiles for the projections
TC = 512           # t-chunk (matmul free dim)
NMC = T // TC      # 4 m-chunks per batch
NST = T // P       # 16 s-tiles per batch
HQ = 4             # heads per core
CB = HQ * D        # 256 channels per core
KB = CB // P       # 2 column blocks per core
MASK_NEG = -30000.0

_CACHE = {}


def _build_program(repeat=1):
    import concourse.bass as bass
    from concourse import bacc
    import concourse.mybir as mybir
    from concourse.tile import TileContext

    dt = mybir.dt
    nc = bacc.Bacc("TRN2", target_bir_lowering=False, debug=False, num_devices=8)

    srct = nc.dram_tensor("srct", [NMC, P, KC, TC], dt.float16, kind="ExternalInput")
    wqkv = nc.dram_tensor("wqkv", [C, 3 * CB], dt.float16, kind="ExternalInput")
    wo = nc.dram_tensor("wo", [P, KB, C], dt.float16, kind="ExternalInput")
    bias = nc.dram_tensor("bias", [P, KB, 3], dt.float32, kind="ExternalInput")
    m0 = nc.dram_tensor("m0", [P, P], dt.float32, kind="ExternalInput")
    ident = nc.dram_tensor("ident", [P, P], dt.bfloat16, kind="ExternalInput")
    ones = nc.dram_tensor("ones", [P, NST], dt.bfloat16, kind="ExternalInput")
    out = nc.dram_tensor("out", [T, C], dt.float16, kind="ExternalOutput")

    wqkv_t = wqkv.ap().rearrange("(ko p) j -> p ko j", p=P)

    ACT_EXP = mybir.ActivationFunctionType.Exp

    with TileContext(nc) as tc:
        with (
            tc.tile_pool(name="persist", bufs=1) as persist,
            tc.tile_pool(name="srcp", bufs=3) as srcp,
            tc.tile_pool(name="vtp", bufs=2) as vtp,
            tc.tile_pool(name="ep", bufs=6) as ep,
            tc.tile_pool(name="nrm", bufs=3) as nrm,
            tc.tile_pool(name="outp", bufs=3) as outp,
            tc.tile_pool(name="psj", bufs=2, space="PSUM") as psj,
            tc.tile_pool(name="pss", bufs=2, space="PSUM") as pss,
            tc.tile_pool(name="pso", bufs=2, space="PSUM") as pso,
            tc.tile_pool(name="psp", bufs=2, space="PSUM") as psp,
        ):
            # first-use order gates the serial DMA-issue stream: src chunk 0
            # + weights unblock the first matmuls
            src0_sb = srcp.tile([P, KC, TC], dt.float16, name="src_sb",
                                tag="src_sb")
            nc.sync.dma_start(out=src0_sb[:], in_=srct.ap()[0])
            wqkv_sb = persist.tile([P, KC, 3 * CB], dt.float16, name="wqkv_sb")
            nc.sync.dma_start(out=wqkv_sb[:], in_=wqkv_t)
            bias_sb = persist.tile([P, KB, 3], dt.float32, name="bias_sb")
            nc.sync.dma_start(out=bias_sb[:], in_=bias.ap())
            ident_sb = persist.tile([P, P], dt.bfloat16, name="ident_sb")
            nc.sync.dma_start(out=ident_sb[:], in_=ident.ap())
            m0_sb = persist.tile([P, P], dt.float32, name="m0_sb")
            ones_sb = persist.tile([P, NST], dt.bfloat16, name="ones_sb")
            wo_sb = persist.tile([P, KB, C], dt.float16, name="wo_sb")

            qT_sb = persist.tile([P, KB, T], dt.float16, name="qT_sb")
            kT_sb = persist.tile([P, KB, T], dt.float16, name="kT_sb")
            # v layout per s-tile: 4 head-major blocks of [v(0:64)|ones]
            v_sb = persist.tile([P, NST, HQ * 65], dt.bfloat16, name="v_sb")
            aoT_sb = persist.tile([P, KB, T], dt.float16, name="aoT_sb")

            def emit_deferred_loads():
                nc.sync.dma_start(out=ones_sb[:], in_=ones.ap())
                for h in range(HQ):
                    nc.vector.tensor_copy(v_sb[:, :, h * 65 + 64], ones_sb[:])
                nc.sync.dma_start(out=m0_sb[:], in_=m0.ap())
                nc.sync.dma_start(out=wo_sb[:], in_=wo.ap())

            def emit_proj(mc, first_src=None):
                msl = slice(mc * TC, (mc + 1) * TC)
                if first_src is not None:
                    src_sb = first_src
                else:
                    src_sb = srcp.tile([P, KC, TC], dt.float16,
                                       name="src_sb", tag="src_sb")
                    nc.sync.dma_start(out=src_sb[:], in_=srct.ap()[mc])

                for kb in range(KB):
                    csl = slice(kb * P, (kb + 1) * P)
                    ps_q = psj.tile([P, TC], dt.float32, name="ps_q", tag="psj")
                    for ko in range(KC):
                        nc.tensor.matmul(
                            ps_q[:], wqkv_sb[:, ko, csl], src_sb[:, ko, :],
                            start=(ko == 0), stop=(ko == KC - 1),
                        )
                    nc.vector.tensor_scalar(
                        qT_sb[:, kb, msl], ps_q[:], bias_sb[:, kb, 0:1], None,
                        mybir.AluOpType.add,
                    )

                    ps_k = psj.tile([P, TC], dt.float32, name="ps_k", tag="psj")
                    for ko in range(KC):
                        nc.tensor.matmul(
                            ps_k[:], wqkv_sb[:, ko, CB + kb * P:CB + (kb + 1) * P],
                            src_sb[:, ko, :],
                            start=(ko == 0), stop=(ko == KC - 1),
                        )
                    nc.vector.tensor_scalar(
                        kT_sb[:, kb, msl], ps_k[:], bias_sb[:, kb, 1:2], None,
                        mybir.AluOpType.add,
                    )

                    ps_v = psj.tile([P, TC], dt.float32, name="ps_v", tag="psj")
                    for ko in range(KC):
                        nc.tensor.matmul(
                            ps_v[:], wqkv_sb[:, ko, 2 * CB + kb * P:2 * CB + (kb + 1) * P],
                            src_sb[:, ko, :],
                            start=(ko == 0), stop=(ko == KC - 1),
                        )
                    vt_sb = vtp.tile([P, TC], dt.bfloat16, name="vt_sb")
                    nc.vector.tensor_scalar(
                        vt_sb[:], ps_v[:], bias_sb[:, kb, 2:3], None,
                        mybir.AluOpType.add,
                    )
                    # transpose the 4 [128,128] blocks of this chunk into v_sb
                    for k4 in range(TC // P):
                        st = mc * (TC // P) + k4
                        ps_t = psp.tile([P, P], dt.bfloat16, name="ps_t", tag="psp")
                        nc.tensor.transpose(
                            ps_t[:], vt_sb[:, k4 * P:(k4 + 1) * P], ident_sb[:]
                        )
                        h0 = 2 * kb
                        nc.vector.tensor_copy(v_sb[:, st, h0 * 65:h0 * 65 + 64],
                                              ps_t[:, 0:64])
                        nc.vector.tensor_copy(
                            v_sb[:, st, (h0 + 1) * 65:(h0 + 1) * 65 + 64],
                            ps_t[:, 64:128])

            def emit_attn(tci):
                t0 = tci * TC
                n_st = (tci + 1) * (TC // P)
                for h in range(HQ):
                    kb = h // 2
                    jh = (h % 2) * D
                    vcol = h * 65
                    ps_o = pso.tile([P, TC], dt.float32, name="ps_o", tag="pso")
                    for st in range(n_st):
                        s0 = st * P
                        k = st - 4 * tci  # >=0 on diagonal blocks
                        toff = max(0, k) * P     # first surviving column
                        L = TC - toff            # surviving width
                        ps_s = pss.tile([P, TC], dt.float32, name="ps_s",
                                        tag="pss")
                        nc.tensor.matmul(
                            ps_s[:, 0:L],
                            kT_sb[jh:jh + D, kb, s0:s0 + P],
                            qT_sb[jh:jh + D, kb, t0 + toff:t0 + TC],
                            start=True, stop=True,
                        )
                        if k >= 0:  # straddling sub-block: mask t<s part
                            nc.vector.tensor_tensor(
                                ps_s[:, 0:P], ps_s[:, 0:P], m0_sb[:],
                                mybir.AluOpType.add,
                            )
                        e_sb = ep.tile([P, TC], dt.bfloat16, name="e_sb")
                        nc.scalar.activation(e_sb[:, 0:L], ps_s[:, 0:L],
                                             ACT_EXP)
                        nc.tensor.matmul(
                            ps_o[0:65, toff:TC],
                            v_sb[:, st, vcol:vcol + 65],
                            e_sb[:, 0:L],
                            start=(st == 0), stop=(st == n_st - 1),
                        )
                    # normalize rows 0..63 by row 64
                    rc_sb = nrm.tile([65, TC], dt.float32, name="rc_sb")
                    nc.vector.reciprocal(rc_sb[64:65, :], ps_o[64:65, :])
                    rc0_sb = nrm.tile([1, TC], dt.float32, name="rc0_sb")
                    nc.gpsimd.dma_start(out=rc0_sb[:], in_=rc_sb[64:65, :])
                    rb_sb = nrm.tile([D, TC], dt.float32, name="rb_sb")
                    nc.gpsimd.partition_broadcast(rb_sb[:], rc0_sb[:])
                    nc.vector.tensor_tensor(
                        aoT_sb[jh:jh + D, kb, t0:t0 + TC],
                        ps_o[0:64, :], rb_sb[:],
                        mybir.AluOpType.mult,
                    )

            def emit_oproj(mts):
                for mt in mts:
                    o_sb = outp.tile([P, C], dt.float16, name="o_sb")
                    for cc in range(C // TC):
                        ps_p = psp.tile([P, TC], dt.float32, name="ps_p",
                                        tag="psp")
                        for kb in range(KB):
                            nc.tensor.matmul(
                                ps_p[:],
                                aoT_sb[:, kb, mt * P:(mt + 1) * P],
                                wo_sb[:, kb, cc * TC:(cc + 1) * TC],
                                start=(kb == 0), stop=(kb == KB - 1),
                            )
                        nc.vector.tensor_copy(o_sb[:, cc * TC:(cc + 1) * TC],
                                              ps_p[:])
                    nc.sync.dma_start(
                        out=out.ap()[mt * P:(mt + 1) * P, :],
                        in_=o_sb[:],
                    )

            first = src0_sb
            for _ in range(repeat):
                # emission order = scheduler priority: keep the PE fed with
                # the next pipeline stage while attention chains drain
                emit_proj(0, first_src=first)
                first = None
                emit_deferred_loads()
                emit_proj(1)
                emit_attn(0)
                emit_proj(2)
                emit_attn(1)
                emit_oproj(range(0, 4))
                emit_proj(3)
                emit_attn(2)
                emit_oproj(range(4, 8))
                emit_attn(3)
                emit_oproj(range(8, 12))
                emit_oproj(range(12, 16))

    nc.compile()
    return nc


def _host_inputs(src, mask, Wq, bq, Wk, bk, Wv, bv, Wo, bo):
    import ml_dtypes

    f32 = np.float32
    f16 = np.float16
    src = np.asarray(src, f32)

    # straddle mask: m0[p, f] = 0 (keep) iff f >= p, else MASK_NEG
    f = np.arange(P)[None, :]
    s = np.arange(P)[:, None]
    m0 = np.where(f >= s, 0.0, MASK_NEG).astype(f32)
    ident = np.eye(P, dtype=ml_dtypes.bfloat16)
    ones_t = np.ones((P, NST), ml_dtypes.bfloat16)

    Wq = np.asarray(Wq, f32) * 0.125   # fold in 1/sqrt(D)
    bq8 = np.asarray(bq, f32) * 0.125
    Wk = np.asarray(Wk, f32)
    Wv = np.asarray(Wv, f32)
    Wo = np.asarray(Wo, f32)

    # per-batch srcT chunks [NMC, P, KC, TC]
    srct_b = []
    for b in range(B):
        st = src[b].T.reshape(KC, P, NMC, TC).transpose(2, 1, 0, 3)
        srct_b.append(np.ascontiguousarray(st, dtype=f16))

    in_maps = []
    for c in range(8):
        b = c // 4
        g = c % 4
        sl = slice(g * CB, (g + 1) * CB)
        wqkv_c = np.concatenate(
            [Wq[:, sl], Wk[:, sl], Wv[:, sl]], axis=1).astype(f16)
        bias_c = np.stack(
            [bq8[sl], np.asarray(bk, f32)[sl], np.asarray(bv, f32)[sl]],
            axis=1,
        ).reshape(KB, P, 3).transpose(1, 0, 2)
        wo_c = Wo[sl, :].reshape(KB, P, C).transpose(1, 0, 2).astype(f16)
        in_maps.append({
            "srct": srct_b[b],
            "wqkv": np.ascontiguousarray(wqkv_c),
            "wo": np.ascontiguousarray(wo_c),
            "bias": np.ascontiguousarray(bias_c.astype(f32)),
            "m0": m0, "ident": ident, "ones": ones_t,
        })
    return in_maps


def kernel(src, mask, Wq, bq, Wk, bk, Wv, bv, Wo, bo):
    from concourse.bass_utils import run_bass_kernel_spmd

    if "nc" not in _CACHE:
        _CACHE["nc"] = _build_program()
    nc = _CACHE["nc"]

    in_maps = _host_inputs(src, mask, Wq, bq, Wk, bk, Wv, bv, Wo, bo)
    res = run_bass_kernel_spmd(nc, in_maps, list(range(8)))

    out = np.zeros((B, T, C), np.float32)
    for c in range(8):
        out[c // 4] += res.results[c]["out"].astype(np.float32)
    out += np.asarray(bo, np.float32)[None, None, :]
    return out
